# revision 2
# baseline (speedup 1.0000x reference)
"""GNN neighborhood aggregation (gather + mean) on 8 TRN2 NeuronCores.

reference:
    idx = concat([nodes[:,None], neigh_idx], 1)   # [B, 33]
    out = features[idx].mean(1)                   # [B, 128]

Strategy: pure data parallelism over the 50k seed nodes (6250 per core);
the 1M x 128 fp32 feature table is replicated on every core.  Per seed
tile of 128 seeds, one indirect DMA gathers all 33*128 rows (each row is
512 B, one descriptor per row) into SBUF laid out [128 part, 33*128
floats]; a log-tree of wide DVE adds reduces the 33 chunks; ACT scales
by 1/33; HWDGE DMA writes the [128, 128] result tile back to DRAM.
"""

import sys

if "/opt/trn_rl_repo" not in sys.path:
    sys.path.insert(0, "/opt/trn_rl_repo")

import numpy as np

N_NODES = 1_000_000
D = 128
B = 50_000
K = 33  # self + 32 neighbors
NCORES = 8
B_LOC = B // NCORES  # 6250
P = 128
NT = (B_LOC + P - 1) // P  # 49
B_PAD = NT * P  # 6272

# Module-level knobs test.py may flip (harness just calls kernel()).
PROFILE = False

_cached_nc = None


def build_program(n_nodes=N_NODES, nt=NT, bufs=4):
    import concourse.bacc as bacc
    import concourse.bass as bass
    import concourse.tile as tile
    from concourse import mybir

    nc = bacc.Bacc(
        "TRN2",
        target_bir_lowering=False,
        debug=False,
        num_devices=NCORES,
    )
    feat = nc.dram_tensor(
        "features", [n_nodes, D], mybir.dt.float32, kind="ExternalInput"
    ).ap()
    idx = nc.dram_tensor(
        "idx", [nt, P, K], mybir.dt.int32, kind="ExternalInput"
    ).ap()
    out = nc.dram_tensor(
        "out", [nt * P, D], mybir.dt.float32, kind="ExternalOutput"
    ).ap()

    with tile.TileContext(nc) as tc:
        with tc.tile_pool(name="sbuf", bufs=bufs) as pool:
            for t in range(nt):
                idx_t = pool.tile([P, K], mybir.dt.int32, tag="idx")
                nc.sync.dma_start(out=idx_t[:], in_=idx[t])
                emb = pool.tile([P, K * D], mybir.dt.float32, tag="emb")
                # HW indirect DMA consumes ONE dynamic offset per partition
                # (streams the out free-width contiguously from that row), so
                # gather one neighbor column j per instruction.
                for j in range(K):
                    nc.gpsimd.indirect_dma_start(
                        out=emb[:, j * D : (j + 1) * D],
                        out_offset=None,
                        in_=feat[:],
                        in_offset=bass.IndirectOffsetOnAxis(
                            ap=idx_t[:, j : j + 1], axis=0
                        ),
                    )
                # 33 chunks of D: fold 16,8,4,2,1 then add the 33rd chunk.
                for w in (16, 8, 4, 2, 1):
                    nc.vector.tensor_add(
                        out=emb[:, : w * D],
                        in0=emb[:, : w * D],
                        in1=emb[:, w * D : 2 * w * D],
                    )
                acc = pool.tile([P, D], mybir.dt.float32, tag="acc")
                nc.vector.tensor_add(
                    out=acc[:], in0=emb[:, :D], in1=emb[:, 32 * D : 33 * D]
                )
                outt = pool.tile([P, D], mybir.dt.float32, tag="outt")
                nc.scalar.mul(outt[:], acc[:], 1.0 / K)
                nc.sync.dma_start(out=out[t * P : (t + 1) * P, :], in_=outt[:])
    nc.compile()
    return nc


def shard_inputs(features, nodes, neigh_idx, n_nodes=N_NODES, nt=NT):
    features = np.ascontiguousarray(np.asarray(features), dtype=np.float32)
    nodes = np.asarray(nodes)
    neigh_idx = np.asarray(neigh_idx)
    b = nodes.shape[0]
    b_loc = b // NCORES
    idx_all = np.concatenate(
        [nodes[:, None].astype(np.int32), neigh_idx.astype(np.int32)], axis=1
    )  # [B, K]
    idx_pad = np.zeros((NCORES, nt * P, K), dtype=np.int32)
    idx_pad[:, :b_loc] = idx_all.reshape(NCORES, b_loc, K)
    return [
        {"features": features, "idx": idx_pad[c].reshape(nt, P, K)}
        for c in range(NCORES)
    ], b_loc


def kernel(features, nodes, neigh_idx):
    global _cached_nc
    from concourse import bass_utils

    in_maps, b_loc = shard_inputs(features, nodes, neigh_idx)
    if _cached_nc is None:
        _cached_nc = build_program()
    res = bass_utils.run_bass_kernel_spmd(
        _cached_nc,
        in_maps,
        core_ids=list(range(NCORES)),
        trace=PROFILE,
        trace_cores=list(range(NCORES)) if PROFILE else None,
    )
    if PROFILE:
        kernel.last_result = res
    out = np.concatenate(
        [res.results[c]["out"][:b_loc] for c in range(NCORES)], axis=0
    )
    return out.astype(np.float32, copy=False)


# revision 6
# speedup vs baseline: 1.0054x; 1.0054x over previous
"""GNN neighborhood aggregation (gather + mean) on 8 TRN2 NeuronCores.

reference:
    idx = concat([nodes[:,None], neigh_idx], 1)   # [B, 33]
    out = features[idx].mean(1)                   # [B, 128]

Strategy: pure data parallelism over the 50k seed nodes (6250 per core);
the 1M x 128 fp32 feature table is replicated on every core.  Per seed
tile of 128 seeds, one indirect DMA gathers all 33*128 rows (each row is
512 B, one descriptor per row) into SBUF laid out [128 part, 33*128
floats]; a log-tree of wide DVE adds reduces the 33 chunks; ACT scales
by 1/33; HWDGE DMA writes the [128, 128] result tile back to DRAM.
"""

import sys

if "/opt/trn_rl_repo" not in sys.path:
    sys.path.insert(0, "/opt/trn_rl_repo")

import numpy as np

N_NODES = 1_000_000
D = 128
B = 50_000
K = 33  # self + 32 neighbors
NCORES = 8
B_LOC = B // NCORES  # 6250
P = 128
NT = (B_LOC + P - 1) // P  # 49
B_PAD = NT * P  # 6272

# Module-level knobs test.py may flip (harness just calls kernel()).
PROFILE = False

_cached_nc = None


def build_program(n_nodes=N_NODES, nt=NT, bufs=6):
    import concourse.bacc as bacc
    import concourse.bass as bass
    import concourse.tile as tile
    from concourse import mybir

    nc = bacc.Bacc(
        "TRN2",
        target_bir_lowering=False,
        debug=False,
        num_devices=NCORES,
    )
    feat = nc.dram_tensor(
        "features", [n_nodes, D], mybir.dt.float32, kind="ExternalInput"
    ).ap()
    # host supplies idx pre-transposed to [P, nt*K] so the preload is one
    # contiguous-per-partition DMA (idx[p, t*K+j] = neighbor j of seed t*P+p)
    idx = nc.dram_tensor(
        "idx", [P, nt * K], mybir.dt.int32, kind="ExternalInput"
    ).ap()
    out = nc.dram_tensor(
        "out", [nt * P, D], mybir.dt.float32, kind="ExternalOutput"
    ).ap()

    with tile.TileContext(nc) as tc:
        with tc.tile_pool(name="idxp", bufs=1) as idxp, tc.tile_pool(
            name="sbuf", bufs=bufs
        ) as pool:
            idx_all = idxp.tile([P, nt * K], mybir.dt.int32, tag="idxall")
            nc.sync.dma_start(out=idx_all[:], in_=idx[:])
            for t in range(nt):
                idx_t = idx_all[:, t * K : (t + 1) * K]
                emb = pool.tile([P, K * D], mybir.dt.float32, tag="emb")
                # HW indirect DMA consumes ONE dynamic offset per partition
                # (streams the out free-width contiguously from that row), so
                # gather one neighbor column j per instruction.
                for j in range(K):
                    nc.gpsimd.indirect_dma_start(
                        out=emb[:, j * D : (j + 1) * D],
                        out_offset=None,
                        in_=feat[:],
                        in_offset=bass.IndirectOffsetOnAxis(
                            ap=idx_t[:, j : j + 1],
                            axis=0,
                        ),
                    )
                # 33 chunks of D: fold 16,8,4,2,1 then add the 33rd chunk.
                for w in (16, 8, 4, 2, 1):
                    nc.vector.tensor_add(
                        out=emb[:, : w * D],
                        in0=emb[:, : w * D],
                        in1=emb[:, w * D : 2 * w * D],
                    )
                acc = pool.tile([P, D], mybir.dt.float32, tag="acc")
                nc.vector.tensor_add(
                    out=acc[:], in0=emb[:, :D], in1=emb[:, 32 * D : 33 * D]
                )
                outt = pool.tile([P, D], mybir.dt.float32, tag="outt")
                nc.scalar.mul(outt[:], acc[:], 1.0 / K)
                nc.sync.dma_start(out=out[t * P : (t + 1) * P, :], in_=outt[:])
    nc.compile()
    return nc


def shard_inputs(features, nodes, neigh_idx, n_nodes=N_NODES, nt=NT):
    features = np.ascontiguousarray(np.asarray(features), dtype=np.float32)
    nodes = np.asarray(nodes)
    neigh_idx = np.asarray(neigh_idx)
    b = nodes.shape[0]
    b_loc = b // NCORES
    idx_all = np.concatenate(
        [nodes[:, None].astype(np.int32), neigh_idx.astype(np.int32)], axis=1
    )  # [B, K]
    idx_pad = np.zeros((NCORES, nt * P, K), dtype=np.int32)
    idx_pad[:, :b_loc] = idx_all.reshape(NCORES, b_loc, K)
    # [nt*P, K] -> [P, nt*K]: idx_t[p, t*K+j] = idx_pad[c, t*P+p, j]
    idx_t = (
        idx_pad.reshape(NCORES, nt, P, K)
        .transpose(0, 2, 1, 3)
        .reshape(NCORES, P, nt * K)
    )
    return [
        {"features": features, "idx": np.ascontiguousarray(idx_t[c])}
        for c in range(NCORES)
    ], b_loc


def kernel(features, nodes, neigh_idx):
    global _cached_nc
    from concourse import bass_utils

    in_maps, b_loc = shard_inputs(features, nodes, neigh_idx)
    if _cached_nc is None:
        _cached_nc = build_program()
    res = bass_utils.run_bass_kernel_spmd(
        _cached_nc,
        in_maps,
        core_ids=list(range(NCORES)),
        trace=PROFILE,
        trace_cores=list(range(NCORES)) if PROFILE else None,
    )
    if PROFILE:
        kernel.last_result = res
    out = np.concatenate(
        [res.results[c]["out"][:b_loc] for c in range(NCORES)], axis=0
    )
    return out.astype(np.float32, copy=False)
